# revision 20
# baseline (speedup 1.0000x reference)
"""Multi-head attention Trainium2 kernel (8 NeuronCores).

Sharding: core c -> (batch b = c//2, head-group hg = c%2 of 8 heads).
Each core computes, for its (b, hg):
  qhT/khT = (Wq/Wk @ x^T) scaled, vh = v @ Wv^T        (fp32 PE matmuls)
  per head: scores -> mask-mul -> exp(+rowsum) -> normalize
            -> attn_proba out (f32), bf16 copy -> xbar DMA transpose
            -> PV matmul (bf16) -> partial out projection (fp32)
Host: pre-transposes x/w (layout prep), sums the two partial output
projections per batch, reassembles full outputs.

Returns (out [4,1024,1024] f32, attn_proba [4,16,1024,1024] f32),
matching the reference's tuple.
"""
import os
import sys

sys.path.insert(0, "/opt/trn_rl_repo")

import ml_dtypes
import numpy as np
from contextlib import ExitStack

import concourse.bass as bass
import concourse.tile as tile
from concourse import mybir
from concourse.bass_utils import run_bass_kernel_spmd

def _install_profile_hook():
    """The boot-time NTFF hook install needs antenv.axon_hooks, which this
    image's antenv lacks. Provide the module and install the hook so
    run_bass_kernel_spmd(trace=True) can report HW exec time."""
    try:
        import types
        import antenv

        if "antenv.axon_hooks" not in sys.modules:
            mod = types.ModuleType("antenv.axon_hooks")
            mod._hook = None
            mod.set_axon_ntff_profile_hook = lambda h: setattr(mod, "_hook", h)
            mod.get_axon_ntff_profile_hook = lambda: mod._hook
            sys.modules["antenv.axon_hooks"] = mod
            antenv.axon_hooks = mod
        from antenv.axon_hooks import (
            get_axon_ntff_profile_hook,
            set_axon_ntff_profile_hook,
        )

        if get_axon_ntff_profile_hook() is None:
            from trn_agent_boot.trn_boot import _ntff_profile_via_ctypes

            set_axon_ntff_profile_hook(
                _ntff_profile_via_ctypes("/opt/axon/libaxon_pjrt.so")
            )
    except Exception:
        pass


_install_profile_hook()

F32 = mybir.dt.float32
F32R = mybir.dt.float32r   # single-pass FP22-precision matmul reads
BF16 = mybir.dt.bfloat16
AL = mybir.AluOpType
AF = mybir.ActivationFunctionType


def _r(ap):
    return ap.bitcast(F32R)


def _bc(ap_tile, n):
    """Broadcast a [128,1] tile along the free axis via stride-0 AP."""
    ap = ap_tile[:]
    return bass.AP(tensor=ap.tensor, offset=ap.offset,
                   ap=[list(ap.ap[0]), [0, n]])

B, S, D = 4, 1024, 1024
H, DH = 16, 64          # heads, head dim
HPC = 8                 # heads per core
DHC = HPC * DH          # 512 head dims per core
N_CORES = 8

LAST_EXEC_NS = None


def _split_sync_waits(nc, max_waits=1):
    """walrus here accepts only one sync-wait per instruction; move excess
    waits onto preceding NoOps on the same engine."""
    f = nc.m.functions[0]
    for blk in f.blocks:
        new_list = []
        for inst in blk.instructions:
            si = inst.sync_info
            if si is not None and len(si.on_wait) > max_waits:
                waits = list(si.on_wait)
                keep = waits[-max_waits:]
                rest = waits[:-max_waits]
                for i in range(0, len(rest), max_waits):
                    nop = mybir.InstNoOp(
                        name=f"{inst.name}-ws{i}", ins=[], outs=[]
                    )
                    nop.engine = inst.engine
                    nop.sync_info = mybir.SyncInfo(
                        on_wait=rest[i:i + max_waits], on_update=[]
                    )
                    nc.register_instruction(nop)
                    new_list.append(nop)
                si.on_wait = keep
            new_list.append(inst)
        blk.instructions[:] = new_list


def _build_program():
    nc = bass.Bass("TRN2", target_bir_lowering=False, debug=False)

    # allow a little more SBUF than the stale 192KB/partition default
    import concourse.tile_utils as tile_utils
    tile_utils.max_sbuf_usage = 192 * 1024

    # ---- per-core DRAM I/O ----
    d = lambda name, shape, dt=F32, out=False: nc.dram_tensor(
        name, shape, dt, kind="ExternalOutput" if out else "ExternalInput"
    ).ap()

    xqT = d("xqT", [8, 128, S])        # q[b]^T as [din/128, 128, s]
    xkT = d("xkT", [8, 128, S])
    xvT = d("xvT", [8, 128, S], BF16)
    mf = d("mf", [8, 128, S], BF16)    # (1 - mask[b]), [q/128, 128, k]
    wqT = d("wqT", [8, 128, DHC])      # w_q[hs]^T as [din/128, 128, dh]
    wkT = d("wkT", [8, 128, DHC])
    wvT = d("wvT", [8, 128, DHC], BF16)
    woT = d("woT", [4, 128, D], BF16)  # w_o[:, hs]^T as [din/128, 128, dout]
    bq8 = d("bq8", [128, 4])           # b_q[hs]/8, partition-major
    bk = d("bk", [128, 4])
    bv = d("bv", [DHC])                # broadcast along partitions on-chip
    bo = d("bo", [128, 8])             # b_o (zeros on hg=1 cores)
    attn_p = d("attn_p", [HPC, S, S], BF16, out=True)
    out_t = d("out_t", [D, S], out=True)   # partial (out @ woT_slice)^T

    with tile.TileContext(nc) as tc:
        with ExitStack() as ctx:
            singles = ctx.enter_context(tc.tile_pool(name="singles", bufs=1))

            # resident tiles
            t_qhT = singles.tile([128, 4, S], F32)    # [dh%128, dh/128, s]
            t_khT = singles.tile([128, 4, S], F32)
            t_vh = singles.tile([128, 8, DHC], BF16)  # [s%128, s/128, dh]
            t_mf = singles.tile([128, 8, S], BF16)    # [q%128, q/128, k]
            t_woT = singles.tile([128, 4, D], BF16)
            t_outT = singles.tile([128, 4, S], BF16)  # attn out, [dh, s]
            t_wq = singles.tile([128, 8, DHC], F32)
            t_wk = singles.tile([128, 8, DHC], F32)
            t_bq8 = singles.tile([128, 4], F32)
            t_bk = singles.tile([128, 4], F32)
            t_bo = singles.tile([128, 8], F32)
            t_bv = singles.tile([128, DHC], F32)      # b_v broadcast

            nc.sync.dma_start(t_mf[:], mf.rearrange("a p c -> p a c"))
            nc.sync.dma_start(t_woT[:], woT.rearrange("a p c -> p a c"))
            nc.sync.dma_start(_r(t_wq[:]), _r(wqT.rearrange("a p c -> p a c")))
            nc.sync.dma_start(_r(t_wk[:]), _r(wkT.rearrange("a p c -> p a c")))
            nc.sync.dma_start(t_bq8[:], bq8[:])
            nc.sync.dma_start(t_bk[:], bk[:])
            nc.sync.dma_start(t_bo[:], bo[:])
            bv_bcast = bass.AP(
                tensor=bv.tensor, offset=bv.offset,
                ap=[[0, 128]] + list(bv.ap),
            )
            nc.gpsimd.dma_start(t_bv[:], bv_bcast)

            # ---- V projection (bf16): vh[s,dh] = xvT[:,st]^T @ wvT ----
            with ExitStack() as pctx:
                xin = pctx.enter_context(tc.tile_pool(name="xin", bufs=3))
                ps_p = pctx.enter_context(
                    tc.tile_pool(name="ps_p", bufs=2, space="PSUM")
                )
                t_wv = xin.tile([128, 8, DHC], BF16, tag="wv", bufs=1)
                nc.sync.dma_start(t_wv[:], wvT.rearrange("a p c -> p a c"))
                t_xv = xin.tile([128, 8, S], BF16, tag="xv", bufs=1)
                nc.sync.dma_start(t_xv[:], xvT.rearrange("a p c -> p a c"))
                for st in range(8):
                    ps = ps_p.tile([128, 512], F32)
                    for dt in range(8):
                        nc.tensor.matmul(
                            ps[:],
                            t_xv[:, dt, st * 128:(st + 1) * 128],
                            t_wv[:, dt, :],
                            start=(dt == 0), stop=(dt == 7),
                        )
                    nc.vector.tensor_tensor(
                        t_vh[:, st, :], ps[:], t_bv[:], AL.add
                    )

            # ---- per head-pair: Q/K projection for this pair's dh tile,
            # then attention. Head 2i sits on partitions 0-63, head 2i+1 on
            # 64-127 (disjoint PE row/col groups run concurrently). ----
            with ExitStack() as hctx:
                xs_pool = hctx.enter_context(tc.tile_pool(name="xs", bufs=2))
                ps_pr = hctx.enter_context(
                    tc.tile_pool(name="ps_pr", bufs=2, space="PSUM")
                )
                ps_s = hctx.enter_context(
                    tc.tile_pool(name="ps_s", bufs=2, space="PSUM")
                )
                ps_o = hctx.enter_context(
                    tc.tile_pool(name="ps_o", bufs=2, space="PSUM")
                )
                sm_pool = hctx.enter_context(tc.tile_pool(name="sm", bufs=2))
                e_pool = hctx.enter_context(tc.tile_pool(name="e", bufs=2))
                l_pool = hctx.enter_context(tc.tile_pool(name="l", bufs=8))
                p16_pool = hctx.enter_context(tc.tile_pool(name="p16", bufs=2))
                pT_pool = hctx.enter_context(tc.tile_pool(name="pT", bufs=2))

                for hp in range(4):
                    ht = hp
                    heads = ((2 * hp, 0), (2 * hp + 1, 64))

                    # Q then K projection for this dh-tile, x streamed by dt
                    for which, x_dram, t_w, t_out in (
                        ("q", xqT, t_wq, t_qhT),
                        ("k", xkT, t_wk, t_khT),
                    ):
                        psc = [ps_pr.tile([128, 512], F32, tag="ps_pr",
                                          name=f"pr_{hp}_{which}_{sc}")
                               for sc in range(2)]
                        for dt in range(8):
                            t_x = xs_pool.tile([128, S], F32, tag="xs",
                                               name=f"x_{hp}_{which}_{dt}")
                            nc.sync.dma_start(_r(t_x[:]), _r(x_dram[dt]))
                            for sc in range(2):
                                nc.tensor.matmul(
                                    psc[sc][:],
                                    _r(t_w[:, dt, ht * 128:(ht + 1) * 128]),
                                    _r(t_x[:, sc * 512:(sc + 1) * 512]),
                                    start=(dt == 0), stop=(dt == 7),
                                )
                        for sc in range(2):
                            if which == "q":
                                nc.scalar.activation(
                                    _r(t_out[:, ht, sc * 512:(sc + 1) * 512]),
                                    psc[sc][:], AF.Identity,
                                    bias=t_bq8[:, ht:ht + 1], scale=0.125,
                                )
                            else:
                                nc.scalar.activation(
                                    _r(t_out[:, ht, sc * 512:(sc + 1) * 512]),
                                    psc[sc][:], AF.Identity,
                                    bias=t_bk[:, ht:ht + 1],
                                )

                    # attention for the two heads of this pair
                    t_p16s = [p16_pool.tile([128, 8, S], BF16, tag="p16",
                                            name=f"p16_{hp}_{i}")
                              for i in range(2)]
                    for qt in range(8):
                        pss = [ps_s.tile([128, S], F32, tag="ps_s",
                                         name=f"pss_{hp}_{qt}_{i}")
                               for i in range(2)]
                        for i, (h, po) in enumerate(heads):
                            qh = t_qhT[po:po + 64, ht, :]
                            kh = t_khT[po:po + 64, ht, :]
                            for kc in range(2):
                                nc.tensor.matmul(
                                    pss[i][:, kc * 512:(kc + 1) * 512],
                                    _r(qh[:, qt * 128:(qt + 1) * 128]),
                                    _r(kh[:, kc * 512:(kc + 1) * 512]),
                                    start=True, stop=True,
                                )
                        for i, (h, po) in enumerate(heads):
                            t_sm = sm_pool.tile([128, S], F32)
                            nc.vector.tensor_tensor(
                                t_sm[:], pss[i][:], t_mf[:, qt, :], AL.mult
                            )
                            t_e = e_pool.tile([128, S], F32)
                            t_l = l_pool.tile([128, 1], F32)
                            nc.scalar.activation(
                                t_e[:], t_sm[:], AF.Exp, accum_out=t_l[:]
                            )
                            t_il = l_pool.tile([128, 1], F32)
                            nc.vector.reciprocal(t_il[:], t_l[:])
                            nc.vector.tensor_tensor(
                                t_p16s[i][:, qt, :], t_e[:], _bc(t_il, S),
                                AL.mult,
                            )
                            nc.sync.dma_start(
                                attn_p[h, qt * 128:(qt + 1) * 128, :],
                                t_p16s[i][:, qt, :],
                            )
                    t_pTs = [pT_pool.tile([128, 8, 8, 128], BF16, tag="pT",
                                          name=f"pT_{hp}_{i}")
                             for i in range(2)]
                    for qt in range(8):
                        for i in range(2):
                            nc.sync.dma_start_transpose(
                                t_pTs[i][:, qt, :, :], t_p16s[i][:, qt, :]
                            )
                    for n in range(2):
                        ps2 = ps_o.tile([128, 512], F32)
                        for st in range(8):
                            for i, (h, po) in enumerate(heads):
                                nc.tensor.matmul(
                                    ps2[po:po + 64, :],
                                    t_vh[:, st, h * 64:(h + 1) * 64],
                                    t_pTs[i][:, 4 * n:4 * n + 4, st, :],
                                    start=(st == 0), stop=(st == 7),
                                    tile_position=(0, po),
                                )
                        nc.any.tensor_copy(
                            t_outT[:, ht, n * 512:(n + 1) * 512], ps2[:]
                        )

            # ---- output projection: out_t[dout, q] = woT^T @ outT + bo ----
            with ExitStack() as fctx:
                ps_f = fctx.enter_context(
                    tc.tile_pool(name="ps_f", bufs=2, space="PSUM")
                )
                fo_pool = fctx.enter_context(tc.tile_pool(name="fo", bufs=3))
                for ot in range(8):
                    for n in range(2):
                        ps = ps_f.tile([128, 512], F32)
                        for dt in range(4):
                            nc.tensor.matmul(
                                ps[:],
                                t_woT[:, dt, ot * 128:(ot + 1) * 128],
                                t_outT[:, dt, n * 512:(n + 1) * 512],
                                start=(dt == 0), stop=(dt == 3),
                            )
                        t_fo = fo_pool.tile([128, 512], F32)
                        nc.scalar.activation(
                            t_fo[:], ps[:], AF.Identity,
                            bias=t_bo[:, ot:ot + 1],
                        )
                        nc.gpsimd.dma_start(
                            out_t[ot * 128:(ot + 1) * 128,
                                  n * 512:(n + 1) * 512],
                            t_fo[:],
                        )

    _split_sync_waits(nc, max_waits=1)
    return nc


_NC = None


def _get_program():
    global _NC
    if _NC is None:
        _NC = _build_program()
    return _NC


def _prep_core_inputs(c, q, k, v, mask, w_q, b_q, w_k, b_k, w_v, b_v, w_o, b_o):
    b, hg = c // 2, c % 2
    hs = slice(hg * DHC, (hg + 1) * DHC)
    f32 = np.float32
    return {
        "xqT": np.ascontiguousarray(q[b].T).reshape(8, 128, S).astype(f32),
        "xkT": np.ascontiguousarray(k[b].T).reshape(8, 128, S).astype(f32),
        "xvT": np.ascontiguousarray(v[b].T).reshape(8, 128, S)
            .astype(ml_dtypes.bfloat16),
        "mf": (1 - mask[b, 0]).reshape(8, 128, S).astype(ml_dtypes.bfloat16),
        "wqT": np.ascontiguousarray(w_q[hs].T).reshape(8, 128, DHC).astype(f32),
        "wkT": np.ascontiguousarray(w_k[hs].T).reshape(8, 128, DHC).astype(f32),
        "wvT": np.ascontiguousarray(w_v[hs].T).reshape(8, 128, DHC)
            .astype(ml_dtypes.bfloat16),
        "woT": np.ascontiguousarray(w_o[:, hs].T).reshape(4, 128, D)
            .astype(ml_dtypes.bfloat16),
        "bq8": np.ascontiguousarray((b_q[hs] / 8).reshape(4, 128).T).astype(f32),
        "bk": np.ascontiguousarray(b_k[hs].reshape(4, 128).T).astype(f32),
        "bv": b_v[hs].astype(f32),
        "bo": (np.ascontiguousarray(b_o.reshape(8, 128).T) if hg == 0
               else np.zeros((128, 8))).astype(f32),
    }


def kernel(q, k, v, mask, w_q, b_q, w_k, b_k, w_v, b_v, w_o, b_o):
    global LAST_EXEC_NS
    q, k, v = np.asarray(q), np.asarray(k), np.asarray(v)
    mask = np.asarray(mask)
    args = (q, k, v, mask, np.asarray(w_q), np.asarray(b_q), np.asarray(w_k),
            np.asarray(b_k), np.asarray(w_v), np.asarray(b_v),
            np.asarray(w_o), np.asarray(b_o))

    nc = _get_program()
    in_maps = [_prep_core_inputs(c, *args) for c in range(N_CORES)]
    trace = os.environ.get("BASS_KERNEL_PROFILE") == "1"
    res = run_bass_kernel_spmd(nc, in_maps, list(range(N_CORES)), trace=trace)
    LAST_EXEC_NS = res.exec_time_ns

    attn = np.empty((B, H, S, S), np.float32)
    out = np.empty((B, S, D), np.float32)
    for b in range(B):
        r0 = res.results[2 * b]
        r1 = res.results[2 * b + 1]
        attn[b, :HPC] = r0["attn_p"].astype(np.float32)
        attn[b, HPC:] = r1["attn_p"].astype(np.float32)
        out[b] = (r0["out_t"] + r1["out_t"]).T
    return out, attn


# revision 36
# speedup vs baseline: 1.7609x; 1.7609x over previous
"""Multi-head attention Trainium2 kernel (8 NeuronCores).

Sharding: core c -> (batch b = c//2, head-group hg = c%2 of 8 heads).
Per core, interleaved per head-pair (head 2i on PE rows/cols 0-63,
head 2i+1 on 64-127, so their matmuls share the array):
  Q/K projections in float32r (single-pass FP22 reads), evacuated to
  bf16 [dh, s] tiles; V projection in bf16.
  scores (bf16) -> mask-mult (DVE, fused PSUM evac; masked scores -> 0
  so exp -> 1.0, matching the reference's -1e-9 fill) -> Exp with
  fused row-sum (ACT) -> normalize (DVE, stride-0 broadcast of 1/l)
  -> bf16 probs -> attn_proba out + xbar DMA transpose -> PV matmul
  -> bf16 out projection partial ([dout, q] layout).
Host: pre-transposes q/k/v/w (layout prep), casts bf16 attn to f32,
sums the two partial output projections per batch and transposes.

Returns (out [4,1024,1024] f32, attn_proba [4,16,1024,1024] f32),
matching the reference's tuple. ~0.37 ms on hardware; max scale-rel
error ~4e-3 on both outputs.
"""
import os
import sys

sys.path.insert(0, "/opt/trn_rl_repo")

import ml_dtypes
import numpy as np
from contextlib import ExitStack

import concourse.bass as bass
import concourse.tile as tile
from concourse import mybir
from concourse.bass_utils import run_bass_kernel_spmd

def _install_profile_hook():
    """The boot-time NTFF hook install needs antenv.axon_hooks, which this
    image's antenv lacks. Provide the module and install the hook so
    run_bass_kernel_spmd(trace=True) can report HW exec time."""
    try:
        import types
        import antenv

        if "antenv.axon_hooks" not in sys.modules:
            mod = types.ModuleType("antenv.axon_hooks")
            mod._hook = None
            mod.set_axon_ntff_profile_hook = lambda h: setattr(mod, "_hook", h)
            mod.get_axon_ntff_profile_hook = lambda: mod._hook
            sys.modules["antenv.axon_hooks"] = mod
            antenv.axon_hooks = mod
        from antenv.axon_hooks import (
            get_axon_ntff_profile_hook,
            set_axon_ntff_profile_hook,
        )

        if get_axon_ntff_profile_hook() is None:
            from trn_agent_boot.trn_boot import _ntff_profile_via_ctypes

            set_axon_ntff_profile_hook(
                _ntff_profile_via_ctypes("/opt/axon/libaxon_pjrt.so")
            )
    except Exception:
        pass


_install_profile_hook()

F32 = mybir.dt.float32
F32R = mybir.dt.float32r   # single-pass FP22-precision matmul reads
BF16 = mybir.dt.bfloat16
AL = mybir.AluOpType
AF = mybir.ActivationFunctionType


def _r(ap):
    return ap.bitcast(F32R)


def _bc(ap_tile, n):
    """Broadcast a [128,1] tile along the free axis via stride-0 AP."""
    ap = ap_tile[:]
    return bass.AP(tensor=ap.tensor, offset=ap.offset,
                   ap=[list(ap.ap[0]), [0, n]])

B, S, D = 4, 1024, 1024
H, DH = 16, 64          # heads, head dim
HPC = 8                 # heads per core
DHC = HPC * DH          # 512 head dims per core
N_CORES = 8

LAST_EXEC_NS = None


def _split_sync_waits(nc, max_waits=1):
    """walrus here accepts only one sync-wait per instruction; move excess
    waits onto preceding NoOps on the same engine."""
    f = nc.m.functions[0]
    for blk in f.blocks:
        new_list = []
        for inst in blk.instructions:
            si = inst.sync_info
            if si is not None and len(si.on_wait) > max_waits:
                waits = list(si.on_wait)
                keep = waits[-max_waits:]
                rest = waits[:-max_waits]
                for i in range(0, len(rest), max_waits):
                    nop = mybir.InstNoOp(
                        name=f"{inst.name}-ws{i}", ins=[], outs=[]
                    )
                    nop.engine = inst.engine
                    nop.sync_info = mybir.SyncInfo(
                        on_wait=rest[i:i + max_waits], on_update=[]
                    )
                    nc.register_instruction(nop)
                    new_list.append(nop)
                si.on_wait = keep
            new_list.append(inst)
        blk.instructions[:] = new_list


def _build_program():
    nc = bass.Bass("TRN2", target_bir_lowering=False, debug=False)

    # allow a little more SBUF than the stale 192KB/partition default
    import concourse.tile_utils as tile_utils
    tile_utils.max_sbuf_usage = 192 * 1024

    # ---- per-core DRAM I/O ----
    d = lambda name, shape, dt=F32, out=False: nc.dram_tensor(
        name, shape, dt, kind="ExternalOutput" if out else "ExternalInput"
    ).ap()

    xqT = d("xqT", [8, 128, S])        # q[b]^T as [din/128, 128, s]
    xkT = d("xkT", [8, 128, S])
    xvT = d("xvT", [8, 128, S], BF16)
    mf = d("mf", [8, 128, S], BF16)    # (1 - mask[b]), [q/128, 128, k]
    wqT = d("wqT", [8, 128, DHC])      # w_q[hs]^T as [din/128, 128, dh]
    wkT = d("wkT", [8, 128, DHC])
    wvT = d("wvT", [8, 128, DHC], BF16)
    woT = d("woT", [4, 128, D], BF16)  # w_o[:, hs]^T as [din/128, 128, dout]
    bq8 = d("bq8", [128, 4])           # b_q[hs]/8, partition-major
    bk = d("bk", [128, 4])
    bv = d("bv", [DHC])                # broadcast along partitions on-chip
    bo = d("bo", [128, 8])             # b_o (zeros on hg=1 cores)
    attn_p = d("attn_p", [HPC, S, S], BF16, out=True)
    out_t = d("out_t", [D, S], out=True)   # partial (out @ woT_slice)^T

    with tile.TileContext(nc) as tc:
        with ExitStack() as ctx:
            singles = ctx.enter_context(tc.tile_pool(name="singles", bufs=1))

            # resident tiles
            t_qh16 = singles.tile([128, 4, S], BF16)  # bf16 qhT/8
            t_kh16 = singles.tile([128, 4, S], BF16)  # bf16 khT
            t_vh = singles.tile([128, 8, DHC], BF16)  # [s%128, s/128, dh]
            t_mf = singles.tile([128, 8, S], BF16)    # [q%128, q/128, k]
            t_woT = singles.tile([128, 4, D], BF16)
            t_outT = singles.tile([128, 4, S], BF16)  # attn out, [dh, s]
            t_bq8 = singles.tile([128, 4], F32)
            t_bk = singles.tile([128, 4], F32)
            t_bo = singles.tile([128, 8], F32)
            t_bv = singles.tile([128, DHC], F32)      # b_v broadcast

            nc.sync.dma_start(t_mf[:], mf.rearrange("a p c -> p a c"))
            nc.sync.dma_start(t_woT[:], woT.rearrange("a p c -> p a c"))
            nc.sync.dma_start(t_bq8[:], bq8[:])
            nc.sync.dma_start(t_bk[:], bk[:])
            nc.sync.dma_start(t_bo[:], bo[:])
            bv_bcast = bass.AP(
                tensor=bv.tensor, offset=bv.offset,
                ap=[[0, 128]] + list(bv.ap),
            )
            nc.gpsimd.dma_start(t_bv[:], bv_bcast)

            # ---- V projection (bf16): vh[s,dh] = xvT[:,st]^T @ wvT ----
            with ExitStack() as pctx:
                xin = pctx.enter_context(tc.tile_pool(name="xin", bufs=3))
                ps_p = pctx.enter_context(
                    tc.tile_pool(name="ps_p", bufs=2, space="PSUM")
                )
                t_wv = xin.tile([128, 8, DHC], BF16, tag="wv", bufs=1)
                nc.sync.dma_start(t_wv[:], wvT.rearrange("a p c -> p a c"))
                t_xv = xin.tile([128, 8, S], BF16, tag="xv", bufs=1)
                nc.sync.dma_start(t_xv[:], xvT.rearrange("a p c -> p a c"))
                for st in range(8):
                    ps = ps_p.tile([128, 512], F32)
                    for dt in range(8):
                        nc.tensor.matmul(
                            ps[:],
                            t_xv[:, dt, st * 128:(st + 1) * 128],
                            t_wv[:, dt, :],
                            start=(dt == 0), stop=(dt == 7),
                        )
                    nc.vector.tensor_tensor(
                        t_vh[:, st, :], ps[:], t_bv[:], AL.add
                    )

            # ---- per head-pair: Q/K projection for this pair's dh tile,
            # then attention. Head 2i sits on partitions 0-63, head 2i+1 on
            # 64-127 (disjoint PE row/col groups run concurrently). ----
            with ExitStack() as hctx:
                xs_pool = hctx.enter_context(tc.tile_pool(name="xs", bufs=4))
                ws_pool = hctx.enter_context(tc.tile_pool(name="ws", bufs=2))
                ps_pr = hctx.enter_context(
                    tc.tile_pool(name="ps_pr", bufs=2, space="PSUM")
                )
                ps_s = hctx.enter_context(
                    tc.tile_pool(name="ps_s", bufs=2, space="PSUM")
                )
                ps_o = hctx.enter_context(
                    tc.tile_pool(name="ps_o", bufs=2, space="PSUM")
                )
                sm_pool = hctx.enter_context(tc.tile_pool(name="sm", bufs=5))
                l_pool = hctx.enter_context(tc.tile_pool(name="l", bufs=16))
                p16_pool = hctx.enter_context(tc.tile_pool(name="p16", bufs=2))
                pT_pool = hctx.enter_context(tc.tile_pool(name="pT", bufs=2))

                for hp in range(4):
                    ht = hp
                    heads = ((2 * hp, 0), (2 * hp + 1, 64))

                    # Q then K projection for this dh-tile, x streamed by dt
                    for which, x_dram, w_dram in (
                        ("q", xqT, wqT),
                        ("k", xkT, wkT),
                    ):
                        t_w = ws_pool.tile([128, 8, 128], F32, tag="ws",
                                           name=f"w_{hp}_{which}")
                        nc.sync.dma_start(
                            _r(t_w[:]),
                            _r(w_dram[:, :, ht * 128:(ht + 1) * 128]
                               .rearrange("a p c -> p a c")),
                        )
                        psc = [ps_pr.tile([128, 512], F32, tag="ps_pr",
                                          name=f"pr_{hp}_{which}_{sc}")
                               for sc in range(2)]
                        for half in range(4):
                            t_x = xs_pool.tile([128, 2, S], F32, tag="xs",
                                               name=f"x_{hp}_{which}_{half}")
                            nc.sync.dma_start(
                                _r(t_x[:]),
                                _r(x_dram[2 * half:2 * half + 2]
                                   .rearrange("a p c -> p a c")),
                            )
                            for dt4 in range(2):
                                dt = 2 * half + dt4
                                for sc in range(2):
                                    nc.tensor.matmul(
                                        psc[sc][:],
                                        _r(t_w[:, dt, :]),
                                        _r(t_x[:, dt4, sc * 512:(sc + 1) * 512]),
                                        start=(dt == 0), stop=(dt == 7),
                                    )
                        for sc in range(2):
                            if which == "q":
                                nc.scalar.activation(
                                    t_qh16[:, ht, sc * 512:(sc + 1) * 512],
                                    psc[sc][:], AF.Identity,
                                    bias=t_bq8[:, ht:ht + 1], scale=0.125,
                                )
                            else:
                                nc.scalar.activation(
                                    t_kh16[:, ht, sc * 512:(sc + 1) * 512],
                                    psc[sc][:], AF.Identity,
                                    bias=t_bk[:, ht:ht + 1],
                                )

                    # attention for the two heads of this pair
                    t_p16s = [p16_pool.tile([128, 8, S], BF16, tag="p16",
                                            name=f"p16_{hp}_{i}")
                              for i in range(2)]
                    for qt in range(8):
                        pss = [ps_s.tile([128, S], F32, tag="ps_s",
                                         name=f"pss_{hp}_{qt}_{i}")
                               for i in range(2)]
                        for i, (h, po) in enumerate(heads):
                            qh16 = t_qh16[po:po + 64, ht, :]
                            kh16 = t_kh16[po:po + 64, ht, :]
                            for kc in range(2):
                                nc.tensor.matmul(
                                    pss[i][:, kc * 512:(kc + 1) * 512],
                                    qh16[:, qt * 128:(qt + 1) * 128],
                                    kh16[:, kc * 512:(kc + 1) * 512],
                                    start=True, stop=True,
                                )
                        for i, (h, po) in enumerate(heads):
                            t_sm = sm_pool.tile([128, S], F32)
                            nc.vector.tensor_tensor(
                                t_sm[:], pss[i][:], t_mf[:, qt, :], AL.mult
                            )
                            t_e = sm_pool.tile([128, S], F32, tag="sm",
                                               name=f"e_{hp}_{qt}_{i}")
                            t_l = l_pool.tile([128, 1], F32)
                            nc.scalar.activation(
                                t_e[:], t_sm[:], AF.Exp, accum_out=t_l[:]
                            )
                            t_il = l_pool.tile([128, 1], F32)
                            nc.vector.reciprocal(t_il[:], t_l[:])
                            if qt % 2 == 0:
                                nc.vector.tensor_tensor(
                                    t_p16s[i][:, qt, :], t_e[:], _bc(t_il, S),
                                    AL.mult,
                                )
                            else:
                                t_lnil = l_pool.tile([128, 1], F32)
                                nc.scalar.activation(
                                    t_lnil[:], t_il[:], AF.Ln
                                )
                                nc.scalar.activation(
                                    t_p16s[i][:, qt, :], t_sm[:], AF.Exp,
                                    bias=t_lnil[:],
                                )
                    for i, (h, po) in enumerate(heads):
                        nc.sync.dma_start(
                            attn_p[h].rearrange("(a p) c -> p a c", p=128),
                            t_p16s[i][:],
                        )
                    t_pTs = [pT_pool.tile([128, 8, 8, 128], BF16, tag="pT",
                                          name=f"pT_{hp}_{i}")
                             for i in range(2)]
                    for i in range(2):
                        nc.sync.dma_start_transpose(
                            t_pTs[i][:], t_p16s[i][:].rearrange("p a c -> p (a c)")
                        )
                    ps2s = [ps_o.tile([128, 512], F32, tag="ps_o",
                                      name=f"ps2_{hp}_{n}")
                            for n in range(2)]
                    for st in range(8):
                        for i, (h, po) in enumerate(heads):
                            for n in range(2):
                                nc.tensor.matmul(
                                    ps2s[n][po:po + 64, :],
                                    t_vh[:, st, h * 64:(h + 1) * 64],
                                    t_pTs[i][:, 4 * n:4 * n + 4, st, :],
                                    start=(st == 0), stop=(st == 7),
                                    tile_position=(0, po),
                                )
                    for n in range(2):
                        nc.any.tensor_copy(
                            t_outT[:, ht, n * 512:(n + 1) * 512], ps2s[n][:]
                        )

            # ---- output projection: out_t[dout, q] = woT^T @ outT + bo ----
            with ExitStack() as fctx:
                ps_f = fctx.enter_context(
                    tc.tile_pool(name="ps_f", bufs=2, space="PSUM")
                )
                fo_pool = fctx.enter_context(tc.tile_pool(name="fo", bufs=3))
                for ot in range(8):
                    pfs = [ps_f.tile([128, 512], F32, tag="ps_f",
                                     name=f"pf_{ot}_{n}")
                           for n in range(2)]
                    for dt in range(4):
                        for n in range(2):
                            nc.tensor.matmul(
                                pfs[n][:],
                                t_woT[:, dt, ot * 128:(ot + 1) * 128],
                                t_outT[:, dt, n * 512:(n + 1) * 512],
                                start=(dt == 0), stop=(dt == 3),
                            )
                    for n in range(2):
                        t_fo = fo_pool.tile([128, 512], F32, tag="fo",
                                            name=f"fo_{ot}_{n}")
                        nc.scalar.activation(
                            t_fo[:], pfs[n][:], AF.Identity,
                            bias=t_bo[:, ot:ot + 1],
                        )
                        nc.gpsimd.dma_start(
                            out_t[ot * 128:(ot + 1) * 128,
                                  n * 512:(n + 1) * 512],
                            t_fo[:],
                        )

    _split_sync_waits(nc, max_waits=1)
    return nc


_NC = None


def _get_program():
    global _NC
    if _NC is None:
        _NC = _build_program()
    return _NC


def _prep_core_inputs(c, q, k, v, mask, w_q, b_q, w_k, b_k, w_v, b_v, w_o, b_o):
    b, hg = c // 2, c % 2
    hs = slice(hg * DHC, (hg + 1) * DHC)
    f32 = np.float32
    return {
        "xqT": np.ascontiguousarray(q[b].T).reshape(8, 128, S).astype(f32),
        "xkT": np.ascontiguousarray(k[b].T).reshape(8, 128, S).astype(f32),
        "xvT": np.ascontiguousarray(v[b].T).reshape(8, 128, S)
            .astype(ml_dtypes.bfloat16),
        "mf": (1 - mask[b, 0]).reshape(8, 128, S).astype(ml_dtypes.bfloat16),
        "wqT": np.ascontiguousarray(w_q[hs].T).reshape(8, 128, DHC).astype(f32),
        "wkT": np.ascontiguousarray(w_k[hs].T).reshape(8, 128, DHC).astype(f32),
        "wvT": np.ascontiguousarray(w_v[hs].T).reshape(8, 128, DHC)
            .astype(ml_dtypes.bfloat16),
        "woT": np.ascontiguousarray(w_o[:, hs].T).reshape(4, 128, D)
            .astype(ml_dtypes.bfloat16),
        "bq8": np.ascontiguousarray((b_q[hs] / 8).reshape(4, 128).T).astype(f32),
        "bk": np.ascontiguousarray(b_k[hs].reshape(4, 128).T).astype(f32),
        "bv": b_v[hs].astype(f32),
        "bo": (np.ascontiguousarray(b_o.reshape(8, 128).T) if hg == 0
               else np.zeros((128, 8))).astype(f32),
    }


def kernel(q, k, v, mask, w_q, b_q, w_k, b_k, w_v, b_v, w_o, b_o):
    global LAST_EXEC_NS
    q, k, v = np.asarray(q), np.asarray(k), np.asarray(v)
    mask = np.asarray(mask)
    args = (q, k, v, mask, np.asarray(w_q), np.asarray(b_q), np.asarray(w_k),
            np.asarray(b_k), np.asarray(w_v), np.asarray(b_v),
            np.asarray(w_o), np.asarray(b_o))

    nc = _get_program()
    in_maps = [_prep_core_inputs(c, *args) for c in range(N_CORES)]
    trace = os.environ.get("BASS_KERNEL_PROFILE") == "1"
    res = run_bass_kernel_spmd(nc, in_maps, list(range(N_CORES)), trace=trace)
    LAST_EXEC_NS = res.exec_time_ns

    attn = np.empty((B, H, S, S), np.float32)
    out = np.empty((B, S, D), np.float32)
    for b in range(B):
        r0 = res.results[2 * b]
        r1 = res.results[2 * b + 1]
        attn[b, :HPC] = r0["attn_p"].astype(np.float32)
        attn[b, HPC:] = r1["attn_p"].astype(np.float32)
        out[b] = (r0["out_t"] + r1["out_t"]).T
    return out, attn
